# revision 10
# baseline (speedup 1.0000x reference)
"""Trainium2 Bass kernel for nn_Anchor_dp: semi-global DP alignment scores + moments.

Full inputs in, full outputs out. Strategy: the DP F[i,j]=max(F[i-1,j-1]+S[i-1,j-1],
F[i-1,j]-g, F[i,j-1]-g) has bounded leftward influence (<=192 cols: 64 diag + 128
horiz, by a score-range argument), so L=262144 splits into 1024 fully independent
blocks (8 cores x 128 partitions), each owning 256 output cols plus a 256-col left
halo (W=512). No inter-core communication needed. Within a row, the max-plus scan
F[u]=max(A[u], F[u-1]-1) is one tensor_tensor_scan instruction per row. Per-row
compute windows shrink as rows advance (influence left edge moves right by 1/row).

The substitution matrix S = pattern_norm^T @ onehot_seq is a 4-entry gather
(~1% of the DP flops) done on host; the device streams it in per row-group
(gpsimd-issued DMAs) overlapped with the vector engine's serial DP chain.
Every same-engine RAW is explicitly semaphore-synced when strict=True (the
race-detector-clean build); strict=False drops intra-engine waits and relies
on the DVE's per-op pipeline drain (validated against the reference on HW).
"""
import sys
import numpy as np

sys.path.insert(0, "/opt/trn_rl_repo")

K = 64            # pattern length (DP rows)
L = 262144        # sequence length
NCORES = 8
P = 128           # partitions per core
OWN = 256         # output cols owned per block
HALO = 256        # left halo cols per block
W = OWN + HALO    # block width
SHW = W + (P - 1) * OWN   # per-core shard width = 33024
CORE_OWN = P * OWN        # 32768
SPAD = -1.0e6     # S value for cols left of sequence start (poisons paths)
GAP = 1.0
LEAKY = 0.001
NEG_INIT = -1.0e30
GROUP = 4         # DP rows per S-DMA group


def _build(bias_val, k=K, p_=P, own=OWN, w=W, shw=SHW, lo_list=None,
           group=GROUP, strict=True):
    from concourse import bass
    import concourse.mybir as mybir

    f32 = mybir.dt.float32
    Alu = mybir.AluOpType

    if lo_list is None:
        lo_list = [k + i for i in range(k)]
    ngroups = (k + group - 1) // group

    nc = bass.Bass(target_bir_lowering=False, debug=False)

    # sdat layout [p_, k, w]: per-partition windows pre-gathered on host so each
    # DMA chunk is (rows-per-group * w * 4) contiguous bytes per partition.
    sdat_ext = nc.declare_dram_parameter("sdat", [p_, k * w], f32, isOutput=False)
    out_ext = nc.declare_dram_parameter("out", [p_, own], f32, isOutput=True)

    with (
        nc.Block() as block,
        nc.semaphore("dma_sem") as dma_sem,
        nc.semaphore("dsem") as dsem,
        nc.semaphore("vsem") as vsem,
        nc.sbuf_tensor("s_all", [p_, k * w], f32) as s_all,
        nc.sbuf_tensor("f0", [p_, w], f32) as f0,
        nc.sbuf_tensor("f1", [p_, w], f32) as f1,
        nc.sbuf_tensor("abuf", [p_, w], f32) as abuf,
        nc.sbuf_tensor("tmp", [p_, w], f32) as tmp,
        nc.sbuf_tensor("neg1", [p_, w], f32) as neg1,
        nc.sbuf_tensor("outsb", [p_, own], f32) as outsb,
    ):

        @block.gpsimd
        def _(g):
            # stream S in, one row-group per DMA, ahead of the DP
            for gi in range(ngroups):
                r0 = gi * group
                nr = min(group, k - r0)
                g.dma_start(
                    bass.AP(s_all, r0 * w, [[k * w, p_], [1, nr * w]]),
                    bass.AP(sdat_ext, r0 * w, [[k * w, p_], [1, nr * w]]),
                ).then_inc(dsem, 16)

        @block.vector
        def _(v):
            vcnt = 0

            def inc(ins):
                nonlocal vcnt
                if strict:
                    ins.then_inc(vsem, 1)
                vcnt += 1
                return ins

            def wait():
                if strict:
                    v.wait_ge(vsem, vcnt)

            inc(v.memset(neg1[:, :], -GAP))
            inc(v.memset(f0[:, :], 0.0))
            fprev, fnew = f0, f1
            for i in range(k):
                lo = lo_list[i]
                srow = s_all[:, i * w: (i + 1) * w]
                if i % group == 0:
                    v.wait_ge(dsem, 16 * (i // group + 1))
                wait()
                # tmp[u] = fprev[u-1] + S[i, u]  (local S index u <-> S col j-1)
                inc(v.tensor_tensor(
                    tmp[:, lo:w], fprev[:, lo - 1: w - 1], srow[:, lo:w], Alu.add
                ))
                wait()
                # A[u] = max(fprev[u] - gap, tmp[u])
                inc(v.scalar_tensor_tensor(
                    abuf[:, lo:w], fprev[:, lo:w], GAP, tmp[:, lo:w],
                    Alu.subtract, Alu.max,
                ))
                wait()
                # fnew[u] = max(A[u], fnew[u-1] - gap)   (max-plus scan)
                inc(v.tensor_tensor_scan(
                    fnew[:, lo:w], neg1[:, lo:w], abuf[:, lo:w], NEG_INIT,
                    Alu.add, Alu.max,
                ))
                fprev, fnew = fnew, fprev
            # epilogue on own region: ch = leakyrelu(F + bias)
            fin = fprev
            wait()
            inc(v.tensor_scalar(
                tmp[:, 0:own], fin[:, w - own: w], float(bias_val), 0.0,
                Alu.add, Alu.max,
            ))
            wait()
            inc(v.tensor_scalar(
                abuf[:, 0:own], fin[:, w - own: w], float(bias_val), 0.0,
                Alu.add, Alu.min,
            ))
            wait()
            v.scalar_tensor_tensor(
                outsb[:, :], abuf[:, 0:own], LEAKY, tmp[:, 0:own],
                Alu.mult, Alu.add,
            ).then_inc(vsem, 1)
            vcnt += 1

        @block.sync
        def _(sync):
            sync.wait_ge(vsem, (3 * k + 5) if strict else 1)
            sync.dma_start(
                bass.AP(out_ext, 0, [[own, p_], [1, own]]),
                bass.AP(outsb, 0, [[own, p_], [1, own]]),
            ).then_inc(dma_sem, 16)
            sync.wait_ge(dma_sem, 16)

    return nc


def _moments(ch: np.ndarray):
    """Host-side cheap epilogue: softmax moments, replicating the reference's
    f32 jax ops exactly (skew is at the f32 noise floor; op-for-op parity
    minimizes divergence)."""
    import jax
    import jax.numpy as jnp

    with jax.default_device(jax.devices("cpu")[0]):
        chj = jnp.asarray(ch)
        l1 = jnp.maximum(jnp.sum(jnp.abs(chj), axis=1, keepdims=True), 1e-12)
        dist = jax.nn.softmax(chj / l1, axis=1)
        n = chj.shape[1]
        x = jnp.arange(n, dtype=chj.dtype)
        mean = jnp.sum(dist * x)
        xd = x - mean
        var = jnp.sum(dist * xd * xd)
        z = xd / jnp.sqrt(var)
        skews = np.asarray(jnp.mean(z ** 3))
        kts = np.asarray(jnp.mean(z ** 4) - 3.0)
    return skews, kts


def _host_s(p_norm: np.ndarray, seq: np.ndarray) -> np.ndarray:
    """S_full[i, j] for padded coords j in [0, HALO + L): pattern-column lookup
    of the one-hot sequence (exactly the reference einsum for one-hot data)."""
    idx = np.argmax(seq, axis=0)          # [L]
    onehot_ok = seq.sum(axis=0)           # 1.0 where a base is set
    s_real = p_norm[idx, :].T * onehot_ok[None, :]  # [K, L]; 0 if all-zero col
    s_full = np.empty((K, HALO + L), np.float32)
    s_full[:, :HALO] = SPAD
    s_full[:, HALO:] = s_real
    return s_full


def kernel(patterns, bias, data, anchor_index, _profile=None, _strict=False):
    patterns = np.asarray(patterns, dtype=np.float32)
    bias = np.asarray(bias, dtype=np.float32)
    data = np.asarray(data, dtype=np.float32)
    a = int(np.asarray(anchor_index))

    p = patterns[a]  # [4, K]
    p = p / np.maximum(np.linalg.norm(p, axis=0, keepdims=True), 1e-12)
    bias_val = float(bias[a])

    s_full = _host_s(p.astype(np.float32), data[0])  # [K, HALO + L]

    # input-aware influence bound: M = sum of per-row pattern maxima caps the
    # total horizontal run any optimal path can fund (CPU-verified exact)
    M = int(np.ceil(float(p.max(axis=0).sum())))
    lo_list = [max(1, min(K + i, 127 + i - M)) for i in range(K)]

    nc = _build(bias_val, lo_list=lo_list, strict=_strict)

    # pre-gather per-partition windows -> [P, K, W] per core (contiguous DMA)
    swv = np.lib.stride_tricks.sliding_window_view(s_full, W, axis=1)  # [K, HL-W+1, W]
    in_maps = []
    for c in range(NCORES):
        starts = c * CORE_OWN + np.arange(P) * OWN
        shard = np.ascontiguousarray(swv[:, starts, :].transpose(1, 0, 2)).reshape(
            P, K * W
        )
        in_maps.append({"sdat": shard})

    from concourse.bass_utils import run_bass_kernel_spmd

    res = run_bass_kernel_spmd(nc, in_maps, core_ids=list(range(NCORES)))
    if _profile is not None:
        _profile[0] = res
    ch = np.concatenate(
        [np.asarray(res.results[c]["out"]).reshape(-1) for c in range(NCORES)]
    )[None, :].astype(np.float32)

    skews, kts = _moments(ch)
    return skews, kts, ch


if __name__ == "__main__":
    print("kernel.py loaded ok")


# revision 12
# speedup vs baseline: 1.1828x; 1.1828x over previous
"""Trainium2 Bass kernel for nn_Anchor_dp: semi-global DP alignment scores + moments.

Full inputs in, full outputs out. Strategy: the DP F[i,j]=max(F[i-1,j-1]+S[i-1,j-1],
F[i-1,j]-g, F[i,j-1]-g) has bounded leftward influence (64 diag + <=M+64 horiz cols,
M = sum of per-row pattern maxima — a score-range argument), so L=262144 splits into
1024 fully independent blocks (8 cores x 128 partitions), each owning 256 output
cols plus a left halo (W=512 buffer; per-row compute windows shrink by 1/row).
No inter-core communication needed.

Ramp transform: G[i,u] = F[i,u] + u turns the within-row max-plus scan
F[u]=max(A[u], F[u-1]-1) into a pure running max G[u]=max(Atilde[u], G[u-1])
(single-ALU feedback loop in tensor_tensor_scan instead of add+max), with the
diagonal term absorbing a host-precomputed S+1. Row recurrence in G-space:
  dg[u] = G_prev[u-1] + (S[i,u]+1);  At[u] = max(G_prev[u]-1, dg[u]);
  G[u] = max(At[u], G[u-1]).
The epilogue subtracts the ramp once.

S = pattern_norm^T @ onehot_seq is a 4-entry gather (~1% of the DP flops) done on
host, pre-windowed per partition, streamed in per row-group (gpsimd-issued DMAs,
per-group semaphores, small first groups) overlapped with the DVE's serial DP.
strict=True adds full same-engine RAW semaphore sync (race-detector-clean build
for the simulator); strict=False relies on the DVE's per-op pipeline drain
(validated against the reference on hardware).
"""
import sys
import numpy as np

sys.path.insert(0, "/opt/trn_rl_repo")

K = 64            # pattern length (DP rows)
L = 262144        # sequence length
NCORES = 8
P = 128           # partitions per core
OWN = 256         # output cols owned per block
HALO = 256        # left halo cols per block
W = OWN + HALO    # block width
SHW = W + (P - 1) * OWN   # per-core shard width = 33024
CORE_OWN = P * OWN        # 32768
SPAD = -1.0e6     # S value for cols left of sequence start (poisons paths)
GAP = 1.0
LEAKY = 0.001
NEG_INIT = -1.0e30
GROUPS = [1, 1, 2, 4] + [8] * 7   # DP rows per S-DMA group (sums to K)


def _build(bias_val, k=K, p_=P, own=OWN, w=W, lo_list=None, groups=None,
           strict=True):
    from concourse import bass
    import concourse.mybir as mybir

    f32 = mybir.dt.float32
    Alu = mybir.AluOpType

    if lo_list is None:
        lo_list = [k + i for i in range(k)]
    if groups is None:
        groups = GROUPS if sum(GROUPS) == k else [min(4, k)] * ((k + 3) // 4)
    assert sum(groups) == k
    # row index -> group index, and group start rows
    g_of_row, gstart = [], []
    r = 0
    for gi, nr in enumerate(groups):
        gstart.append(r)
        g_of_row += [gi] * nr
        r += nr

    nc = bass.Bass(target_bir_lowering=False, debug=False)

    # sdat layout [p_, k*w]: per-partition windows pre-gathered on host so each
    # DMA chunk is (rows-per-group * w * 4) contiguous bytes per partition.
    # Row i of the window holds S[i, col]+1 (G-space diagonal term).
    sdat_ext = nc.declare_dram_parameter("sdat", [p_, k * w], f32, isOutput=False)
    ramp_ext = nc.declare_dram_parameter("ramp", [p_, w], f32, isOutput=False)
    out_ext = nc.declare_dram_parameter("out", [p_, own], f32, isOutput=True)

    dsems = []
    with (
        nc.Block() as block,
        nc.semaphore("dma_sem") as dma_sem,
        nc.semaphore("rsem") as rsem,
        nc.semaphore("vsem") as vsem,
        nc.sbuf_tensor("s_all", [p_, k * w], f32) as s_all,
        nc.sbuf_tensor("ramp_sb", [p_, w], f32) as ramp,
        nc.sbuf_tensor("f0", [p_, w], f32) as f0,
        nc.sbuf_tensor("f1", [p_, w], f32) as f1,
        nc.sbuf_tensor("abuf", [p_, w], f32) as abuf,
        nc.sbuf_tensor("tmp", [p_, w], f32) as tmp,
        nc.sbuf_tensor("outsb", [p_, own], f32) as outsb,
    ):
        for gi in range(len(groups)):
            dsems.append(nc.ctx.enter_context(nc.semaphore(f"dsem{gi}")))

        @block.gpsimd
        def _(g):
            # ramp first (f0 init = ramp), then stream S in per row-group
            g.dma_start(
                bass.AP(ramp, 0, [[w, p_], [1, w]]),
                bass.AP(ramp_ext, 0, [[w, p_], [1, w]]),
            ).then_inc(rsem, 16)
            for gi, nr in enumerate(groups):
                r0 = gstart[gi]
                g.dma_start(
                    bass.AP(s_all, r0 * w, [[k * w, p_], [1, nr * w]]),
                    bass.AP(sdat_ext, r0 * w, [[k * w, p_], [1, nr * w]]),
                ).then_inc(dsems[gi], 16)

        @block.vector
        def _(v):
            vcnt = 0

            def inc(ins):
                nonlocal vcnt
                if strict:
                    ins.then_inc(vsem, 1)
                vcnt += 1
                return ins

            def wait():
                if strict:
                    v.wait_ge(vsem, vcnt)

            # G_0[u] = F_0[u] + u = ramp
            v.wait_ge(rsem, 16)
            inc(v.tensor_scalar_add(f0[:, :], ramp[:, :], 0.0))
            fprev, fnew = f0, f1
            for i in range(k):
                lo = lo_list[i]
                srow = s_all[:, i * w: (i + 1) * w]
                if i == 0 or g_of_row[i] != g_of_row[i - 1]:
                    v.wait_ge(dsems[g_of_row[i]], 16)
                wait()
                # dg[u] = G_prev[u-1] + (S[i,u]+1)
                inc(v.tensor_tensor(
                    tmp[:, lo:w], fprev[:, lo - 1: w - 1], srow[:, lo:w], Alu.add
                ))
                wait()
                # At[u] = max(G_prev[u] - 1, dg[u])
                inc(v.scalar_tensor_tensor(
                    abuf[:, lo:w], fprev[:, lo:w], GAP, tmp[:, lo:w],
                    Alu.subtract, Alu.max,
                ))
                wait()
                # G[u] = max(At[u], G[u-1])  (running max, 1-ALU feedback)
                inc(v.tensor_tensor_scan(
                    fnew[:, lo:w], abuf[:, lo:w], abuf[:, lo:w], NEG_INIT,
                    Alu.max, Alu.max,
                ))
                fprev, fnew = fnew, fprev
            # epilogue on own region: ch = leakyrelu(G - ramp + bias)
            fin = fprev
            wait()
            inc(v.tensor_tensor(
                tmp[:, 0:own], fin[:, w - own: w], ramp[:, w - own: w],
                Alu.subtract,
            ))
            wait()
            inc(v.tensor_scalar(
                abuf[:, 0:own], tmp[:, 0:own], float(bias_val), 0.0,
                Alu.add, Alu.max,
            ))
            wait()
            inc(v.tensor_scalar(
                tmp[:, 0:own], tmp[:, 0:own], float(bias_val), 0.0,
                Alu.add, Alu.min,
            ))
            wait()
            v.scalar_tensor_tensor(
                outsb[:, :], tmp[:, 0:own], LEAKY, abuf[:, 0:own],
                Alu.mult, Alu.add,
            ).then_inc(vsem, 1)
            vcnt += 1

        @block.sync
        def _(sync):
            sync.wait_ge(vsem, (3 * k + 5) if strict else 1)
            sync.dma_start(
                bass.AP(out_ext, 0, [[own, p_], [1, own]]),
                bass.AP(outsb, 0, [[own, p_], [1, own]]),
            ).then_inc(dma_sem, 16)
            sync.wait_ge(dma_sem, 16)

    return nc


def _moments(ch: np.ndarray):
    """Host-side cheap epilogue: softmax moments, replicating the reference's
    f32 jax ops exactly (skew is at the f32 noise floor; op-for-op parity
    minimizes divergence)."""
    import jax
    import jax.numpy as jnp

    with jax.default_device(jax.devices("cpu")[0]):
        chj = jnp.asarray(ch)
        l1 = jnp.maximum(jnp.sum(jnp.abs(chj), axis=1, keepdims=True), 1e-12)
        dist = jax.nn.softmax(chj / l1, axis=1)
        n = chj.shape[1]
        x = jnp.arange(n, dtype=chj.dtype)
        mean = jnp.sum(dist * x)
        xd = x - mean
        var = jnp.sum(dist * xd * xd)
        z = xd / jnp.sqrt(var)
        skews = np.asarray(jnp.mean(z ** 3))
        kts = np.asarray(jnp.mean(z ** 4) - 3.0)
    return skews, kts


def _host_s(p_norm: np.ndarray, seq: np.ndarray) -> np.ndarray:
    """S_full[i, j]+1 for padded coords j in [0, HALO + L): pattern-column
    lookup of the one-hot sequence (the reference einsum, exactly, for one-hot
    data), plus the G-space +1."""
    idx = np.argmax(seq, axis=0)          # [L]
    onehot_ok = seq.sum(axis=0)           # 1.0 where a base is set
    s_real = p_norm[idx, :].T * onehot_ok[None, :]  # [K, L]; 0 if all-zero col
    s_full = np.empty((K, HALO + L), np.float32)
    s_full[:, :HALO] = SPAD
    s_full[:, HALO:] = s_real + np.float32(1.0)
    return s_full


def kernel(patterns, bias, data, anchor_index, _profile=None, _strict=False):
    patterns = np.asarray(patterns, dtype=np.float32)
    bias = np.asarray(bias, dtype=np.float32)
    data = np.asarray(data, dtype=np.float32)
    a = int(np.asarray(anchor_index))

    p = patterns[a]  # [4, K]
    p = p / np.maximum(np.linalg.norm(p, axis=0, keepdims=True), 1e-12)
    bias_val = float(bias[a])

    s_full = _host_s(p.astype(np.float32), data[0])  # [K, HALO + L], = S+1

    # input-aware influence bound: M = sum of per-row pattern maxima caps the
    # total horizontal run any optimal path can fund (CPU-verified exact)
    M = int(np.ceil(float(p.max(axis=0).sum())))
    lo_list = [max(1, min(K + i, 127 + i - M)) for i in range(K)]

    nc = _build(bias_val, lo_list=lo_list, strict=_strict)

    # pre-gather per-partition windows -> [P, K, W] per core (contiguous DMA)
    swv = np.lib.stride_tricks.sliding_window_view(s_full, W, axis=1)  # [K, HL-W+1, W]
    ramp = np.broadcast_to(
        np.arange(W, dtype=np.float32)[None, :], (P, W)
    ).copy()
    in_maps = []
    for c in range(NCORES):
        starts = c * CORE_OWN + np.arange(P) * OWN
        shard = np.ascontiguousarray(swv[:, starts, :].transpose(1, 0, 2)).reshape(
            P, K * W
        )
        in_maps.append({"sdat": shard, "ramp": ramp})

    from concourse.bass_utils import run_bass_kernel_spmd

    res = run_bass_kernel_spmd(nc, in_maps, core_ids=list(range(NCORES)))
    if _profile is not None:
        _profile[0] = res
    ch = np.concatenate(
        [np.asarray(res.results[c]["out"]).reshape(-1) for c in range(NCORES)]
    )[None, :].astype(np.float32)

    skews, kts = _moments(ch)
    return skews, kts, ch


if __name__ == "__main__":
    print("kernel.py loaded ok")
